# revision 6
# baseline (speedup 1.0000x reference)
"""AfmoeMoE Trainium2 kernel: 8-core expert-parallel MoE with shared expert.

Reference computation (T=2048, H=1024, FF=512, E=16, top-4):
  scores = sigmoid(x @ gate_w.T); top4 by (scores + bias); renormalize
  routed = sum_e cw[t,e] * (silu(x@w1g[e].T) * (x@w1u[e].T)) @ w2[e].T
  out = routed + shared SwiGLU MLP

Sharding (inside kernel()):
  - expert-parallel: 2 experts per core (w1/w2 sliced on host)
  - shared expert tensor-parallel over FF (64 f-rows per core)
  - router replicated (gate columns permuted per core so the core's own
    2 experts land in columns 0,1 - avoids dynamic indexing)
  - each core computes a partial [H, Tc] output chunk (feature-major),
    ReduceScatter sums over cores and leaves core r with h-rows
    [128r:128(r+1)]; host concatenates shards and transposes.

Dataflow on device is feature-major ([feature-part, token-free]) end to
end, so the expert path needs no transposes. Expert matmuls run in
float32r (PE full rate); the router matmul stays exact float32 because
the 4th/5th expert score gap can be ~1e-5.
"""

import numpy as np

import concourse.bass as bass
import concourse.mybir as mybir
import concourse.tile as tile
from concourse import bacc
from concourse.bass_utils import run_bass_kernel_spmd
from concourse.masks import make_identity

F32 = mybir.dt.float32
F32R = mybir.dt.float32r
AF = mybir.ActivationFunctionType

T, H, FF, E, TOPK = 2048, 1024, 512, 16, 4
NCORES, EPC = 8, 2  # cores, experts per core
TC = 512  # token chunk
KT = H // 128  # contraction tiles over H
FT = FF // 128  # contraction tiles over FF
HT = H // 128  # output tiles over H
NEG = -1.0e30


def moe_program(tc_ctx, io, n_tok):
    nc = tc_ctx.nc
    tc = tc_ctx
    nch = n_tok // TC
    nsub = TC // 128

    with (
        tc.tile_pool(name="const", bufs=1) as constp,
        tc.tile_pool(name="wpool", bufs=1) as wp_pool,
        tc.tile_pool(name="xtp", bufs=1) as xtp,
        tc.tile_pool(name="xtrp", bufs=1) as xtrp,
        tc.tile_pool(name="rpool", bufs=2) as rpool,
        tc.tile_pool(name="cwp", bufs=2) as cwp,
        tc.tile_pool(name="silup", bufs=2) as silup,
        tc.tile_pool(name="actp", bufs=1) as actp,
        tc.tile_pool(name="actshp", bufs=2) as actshp,
        tc.tile_pool(name="stagep", bufs=2) as stagep,
        tc.tile_pool(name="pgu", bufs=2, space="PSUM") as pgu,
        tc.tile_pool(name="psgu", bufs=2, space="PSUM") as psgu,
        tc.tile_pool(name="pflex", bufs=2, space="PSUM") as pflex,
        tc.tile_pool(name="dramp", bufs=2, space="DRAM") as dramp,
    ):
        # ---- constants / weights (resident) ----
        ident = constp.tile([128, 128], F32)
        make_identity(nc, ident[:])
        onesb = constp.tile([128, 128], F32)
        nc.vector.memset(onesb[:], 1.0)
        gate_sb = constp.tile([128, KT, E], F32)
        nc.sync.dma_start(gate_sb[:], io["gate"])
        bias_sb = constp.tile([128, E], F32)
        nc.sync.dma_start(bias_sb[0:1, :], io["biasp"])

        w1sb = []
        w2sb = []
        for e in range(EPC):
            w1t = wp_pool.tile([128, KT, 8, 128], F32R, name=f"w1sb{e}")
            nc.sync.dma_start(w1t[:], io["w1t"][e].bitcast(F32R))
            w1sb.append(w1t)
            w2t = wp_pool.tile([128, FT, HT, 128], F32R, name=f"w2sb{e}")
            nc.sync.dma_start(w2t[:], io["w2t"][e].bitcast(F32R))
            w2sb.append(w2t)
        sw1sb = wp_pool.tile([128, KT, 2, 64], F32R)
        nc.sync.dma_start(sw1sb[:], io["sw1t"].bitcast(F32R))
        sw2sb = wp_pool.tile([128, HT, 128], F32R)
        nc.sync.dma_start(sw2sb[0:64, :, :], io["sw2t"].bitcast(F32R))

        # bias broadcast [1,E] -> [128,E]
        bps = pflex.tile([128, TC], F32, name="bps", tag="flex")
        nc.tensor.matmul(
            bps[:, 0:E], onesb[0:1, :], bias_sb[0:1, :], start=True, stop=True
        )
        biasbc = constp.tile([128, E], F32)
        nc.scalar.copy(biasbc[:], bps[:, 0:E])

        for c in range(nch):
            # ---- load x chunk (transposed layout [h, t]) ----
            xt32 = xtp.tile([128, KT, TC], F32, name="xt32", tag="xt32")
            nc.sync.dma_start(xt32[:], io["xt"][:, :, c * TC : (c + 1) * TC])
            xtr = xtrp.tile([128, KT, TC], F32R, name="xtr", tag="xtr")
            nc.scalar.copy(xtr[:], xt32[:])

            # ---- router (exact fp32) ----
            rps_full = pflex.tile([128, TC], F32, name="rps", tag="flex")
            rps = rps_full[0:16, :]
            for kt in range(KT):
                nc.tensor.matmul(
                    rps,
                    gate_sb[:, kt, :],
                    xt32[:, kt, :],
                    start=(kt == 0),
                    stop=(kt == KT - 1),
                )
            scoresTb = rpool.tile([128, TC], F32, name="scoresT", tag="scoresT")
            scoresT = scoresTb[0:16, :]
            nc.scalar.activation(scoresT, rps, AF.Sigmoid)

            cwTbs = [
                cwp.tile([128, TC], F32, name=f"cwT{e}", tag=f"cwT{e}")
                for e in range(EPC)
            ]
            for s in range(nsub):
                tps_full = pflex.tile([128, TC], F32, name="tps", tag="flex")
                tps = tps_full[:, 0:16]
                nc.tensor.transpose(
                    tps, scoresT[:, s * 128 : (s + 1) * 128], ident[0:16, 0:16]
                )
                scores = rpool.tile([128, E], F32, name="scores", tag="scores")
                nc.scalar.copy(scores[:], tps)
                biased = rpool.tile([128, E], F32, name="biased", tag="biased")
                nc.vector.tensor_add(biased[:], scores[:], biasbc[:])
                t8 = rpool.tile([128, 8], F32, name="t8", tag="t8")
                nc.vector.max(out=t8[:], in_=biased[:])
                nc.vector.memset(t8[:, TOPK:], NEG)
                zap = rpool.tile([128, E], F32, name="zap", tag="zap")
                nc.vector.match_replace(
                    out=zap[:], in_to_replace=t8[:], in_values=biased[:], imm_value=NEG
                )
                sel = rpool.tile([128, E], F32, name="sel", tag="sel")
                nc.vector.tensor_tensor(
                    sel[:], biased[:], zap[:], mybir.AluOpType.not_equal
                )
                cwu = rpool.tile([128, E], F32, name="cwu", tag="cwu")
                nc.vector.tensor_mul(cwu[:], scores[:], sel[:])
                den = rpool.tile([128, 1], F32, name="den", tag="den")
                nc.vector.reduce_sum(den[:], cwu[:], axis=mybir.AxisListType.X)
                rden = rpool.tile([128, 1], F32, name="rden", tag="rden")
                nc.vector.reciprocal(rden[:], den[:])
                cw = rpool.tile([128, EPC], F32, name="cw", tag="cw")
                nc.vector.tensor_scalar_mul(cw[:], cwu[:, 0:EPC], rden[:])
                for e in range(EPC):
                    ctp_full = pflex.tile([128, TC], F32, name="ctp", tag="flex")
                    ctp = ctp_full[0:1, 0:128]
                    nc.tensor.transpose(ctp, cw[:, e : e + 1], ident[:])
                    nc.scalar.copy(cwTbs[e][0:1, s * 128 : (s + 1) * 128], ctp)

            # cw broadcast across partitions via k=1 matmul
            cwbc = cwp.tile([128, EPC, TC], F32, name="cwbc", tag="cwbc")
            for e in range(EPC):
                bc = pflex.tile([128, TC], F32, name="bc", tag="flex")
                nc.tensor.matmul(
                    bc[:], onesb[0:1, :], cwTbs[e][0:1, :], start=True, stop=True
                )
                nc.scalar.copy(cwbc[:, e, :], bc[:])

            # ---- experts: w1 + swiglu (+ cw scale) ----
            actbs = []
            for e in range(EPC):
                actb = actp.tile([128, FT, TC], F32R, name=f"actb{e}", tag=f"actb{e}")
                for j in range(FT):
                    gup = pgu.tile([128, 2, TC], F32, name="gup", tag="gu")
                    for half in range(2):
                        for kt in range(KT):
                            nc.tensor.matmul(
                                gup[:, half, :],
                                w1sb[e][:, kt, 2 * j + half, :],
                                xtr[:, kt, :],
                                start=(kt == 0),
                                stop=(kt == KT - 1),
                            )
                    sg = silup.tile([128, TC], F32, name="sg", tag="sg")
                    nc.scalar.activation(sg[:], gup[:, 0, :], AF.Sigmoid)
                    nc.vector.tensor_mul(sg[:], sg[:], gup[:, 0, :])
                    usc = silup.tile([128, TC], F32, name="usc", tag="usc")
                    nc.vector.tensor_mul(usc[:], gup[:, 1, :], cwbc[:, e, :])
                    nc.vector.tensor_mul(actb[:, j, :], sg[:], usc[:])
                actbs.append(actb)

            # ---- shared expert (TP slice: 64 f-rows) ----
            sgug_f = psgu.tile([128, TC], F32, name="sgug", tag="sgu")
            sguu_f = psgu.tile([128, TC], F32, name="sguu", tag="sgu")
            sgug, sguu = sgug_f[0:64, :], sguu_f[0:64, :]
            for kt in range(KT):
                nc.tensor.matmul(
                    sgug,
                    sw1sb[:, kt, 0, :],
                    xtr[:, kt, :],
                    start=(kt == 0),
                    stop=(kt == KT - 1),
                )
            for kt in range(KT):
                nc.tensor.matmul(
                    sguu,
                    sw1sb[:, kt, 1, :],
                    xtr[:, kt, :],
                    start=(kt == 0),
                    stop=(kt == KT - 1),
                )
            sgs_f = silup.tile([128, TC], F32, name="sgs", tag="sgs")
            sgs = sgs_f[0:64, :]
            nc.scalar.activation(sgs, sgug, AF.Sigmoid)
            nc.vector.tensor_mul(sgs, sgs, sgug)
            actsh_f = actshp.tile([128, TC], F32R, name="actsh", tag="actsh")
            actsh = actsh_f[0:64, :]
            nc.vector.tensor_mul(actsh, sgs, sguu)

            # ---- w2: accumulate both experts + shared into partial [H, TC] ----
            partial = dramp.tile([H, TC], F32, name="partial", tag="partial")
            for ht in range(HT):
                wps = pflex.tile([128, TC], F32, name="wps", tag="flex")
                nk = 0
                for e in range(EPC):
                    for ft in range(FT):
                        nc.tensor.matmul(
                            wps[:],
                            w2sb[e][:, ft, ht, :],
                            actbs[e][:, ft, :],
                            start=(nk == 0),
                            stop=False,
                        )
                        nk += 1
                nc.tensor.matmul(
                    wps[:], sw2sb[0:64, ht, :], actsh, start=False, stop=True
                )
                stg = stagep.tile([128, TC], F32, name="stg", tag="stg")
                nc.scalar.copy(stg[:], wps[:])
                nc.sync.dma_start(partial[ht * 128 : (ht + 1) * 128, :], stg[:])

            # ---- combine across cores ----
            rso = dramp.tile([128, TC], F32, name="rso", tag="rso")
            nc.gpsimd.collective_compute(
                "ReduceScatter",
                mybir.AluOpType.add,
                replica_groups=[list(range(NCORES))],
                ins=[partial.opt()],
                outs=[rso.opt()],
            )
            nc.sync.dma_start(io["out"][:, c * TC : (c + 1) * TC], rso[:])


def build_nc(n_tok=T):
    nc = bacc.Bacc("TRN2", target_bir_lowering=False, debug=False, num_devices=NCORES)
    io = {
        "xt": nc.dram_tensor("xt", [128, KT, n_tok], F32, kind="ExternalInput").ap(),
        "gate": nc.dram_tensor("gate", [128, KT, E], F32, kind="ExternalInput").ap(),
        "biasp": nc.dram_tensor("biasp", [1, E], F32, kind="ExternalInput").ap(),
        "w1t": [
            nc.dram_tensor(f"w1t{e}", [128, KT, 8, 128], F32, kind="ExternalInput").ap()
            for e in range(EPC)
        ],
        "w2t": [
            nc.dram_tensor(f"w2t{e}", [128, FT, HT, 128], F32, kind="ExternalInput").ap()
            for e in range(EPC)
        ],
        "sw1t": nc.dram_tensor("sw1t", [128, KT, 2, 64], F32, kind="ExternalInput").ap(),
        "sw2t": nc.dram_tensor("sw2t", [64, HT, 128], F32, kind="ExternalInput").ap(),
        "out": nc.dram_tensor("out", [128, n_tok], F32, kind="ExternalOutput").ap(),
    }
    with tile.TileContext(nc) as tcx:
        moe_program(tcx, io, n_tok)
    nc.compile()
    return nc


def _sb_layout_kxm(a):
    """[Ktot, M] -> sbuf image [128, KT', M] with h-tiles on axis 1."""
    ktot, m = a.shape
    return np.ascontiguousarray(
        a.reshape(ktot // 128, 128, m).transpose(1, 0, 2)
    )


def prep_inputs(inputs, n_tok=T):
    """Host-side sharding/layout: returns per-core in_maps."""
    x = np.asarray(inputs["hidden_states"], dtype=np.float32)[:n_tok]
    gate_w = np.asarray(inputs["gate_w"], dtype=np.float32)
    ebias = np.asarray(inputs["expert_bias"], dtype=np.float32)
    w1 = np.asarray(inputs["w1"], dtype=np.float32)
    w2 = np.asarray(inputs["w2"], dtype=np.float32)
    sw1 = np.asarray(inputs["shared_w1"], dtype=np.float32)
    sw2 = np.asarray(inputs["shared_w2"], dtype=np.float32)

    xt = _sb_layout_kxm(np.ascontiguousarray(x.T))  # [128, KT, n_tok]

    in_maps = []
    for r in range(NCORES):
        perm = [2 * r, 2 * r + 1] + [e for e in range(E) if e not in (2 * r, 2 * r + 1)]
        gperm = gate_w[perm].T  # [H, E]
        m = {
            "xt": xt,
            "gate": _sb_layout_kxm(gperm).copy(),
            "biasp": ebias[perm][None, :].copy(),
            "sw1t": _sb_layout_kxm(
                np.concatenate(
                    [sw1.T[:, 64 * r : 64 * r + 64], sw1.T[:, FF + 64 * r : FF + 64 * r + 64]],
                    axis=1,
                )
            ).reshape(128, KT, 2, 64),
            "sw2t": np.ascontiguousarray(sw2.T[64 * r : 64 * r + 64]).reshape(
                64, HT, 128
            ),
        }
        for i, e in enumerate((2 * r, 2 * r + 1)):
            w1te = w1[e].T  # [H, 2FF] cols: g 0..FF-1, u FF..2FF-1
            inter = np.empty_like(w1te)
            for j in range(FT):
                inter[:, 256 * j : 256 * j + 128] = w1te[:, 128 * j : 128 * j + 128]
                inter[:, 256 * j + 128 : 256 * j + 256] = w1te[
                    :, FF + 128 * j : FF + 128 * j + 128
                ]
            m[f"w1t{i}"] = _sb_layout_kxm(inter).reshape(128, KT, 8, 128)
            m[f"w2t{i}"] = _sb_layout_kxm(np.ascontiguousarray(w2[e].T)).reshape(
                128, FT, HT, 128
            )
        in_maps.append(m)
    return in_maps


def assemble_output(results, n_tok=T):
    shards = [results[r]["out"] for r in range(NCORES)]
    full = np.concatenate(shards, axis=0)  # [H, n_tok]
    return np.ascontiguousarray(full.T).astype(np.float32)


_NC_CACHE = {}


def kernel(**inputs):
    n_tok = int(np.asarray(inputs["hidden_states"]).shape[0])
    if n_tok not in _NC_CACHE:
        _NC_CACHE[n_tok] = build_nc(n_tok)
    nc = _NC_CACHE[n_tok]
    in_maps = prep_inputs(inputs, n_tok)
    res = run_bass_kernel_spmd(nc, in_maps, core_ids=list(range(NCORES)))
    return assemble_output(res.results, n_tok)


if __name__ == "__main__":
    import reference  # only when run manually alongside reference.py

    inp = reference.setup_inputs()
    out = kernel(**{k: np.asarray(v) for k, v in inp.items()})
    print("out", out.shape, out.dtype)


# revision 7
# speedup vs baseline: 1.0695x; 1.0695x over previous
"""AfmoeMoE Trainium2 kernel: 8-core expert-parallel MoE with shared expert.

Reference computation (T=2048, H=1024, FF=512, E=16, top-4):
  scores = sigmoid(x @ gate_w.T); top4 by (scores + bias); renormalize
  routed = sum_e cw[t,e] * (silu(x@w1g[e].T) * (x@w1u[e].T)) @ w2[e].T
  out = routed + shared SwiGLU MLP

Sharding (inside kernel()):
  - expert-parallel: 2 experts per core (w1/w2 sliced on host)
  - shared expert tensor-parallel over FF (64 f-rows per core)
  - router replicated (gate columns permuted per core so the core's own
    2 experts land in columns 0,1 - avoids dynamic indexing)
  - each core computes a partial [H, Tc] output chunk (feature-major),
    ReduceScatter sums over cores and leaves core r with h-rows
    [128r:128(r+1)]; host concatenates shards and transposes.

Dataflow on device is feature-major ([feature-part, token-free]) end to
end, so the expert path needs no transposes. Expert matmuls run in
float32r (PE full rate); the router matmul stays exact float32 because
the 4th/5th expert score gap can be ~1e-5.
"""

import numpy as np

import concourse.bass as bass
import concourse.mybir as mybir
import concourse.tile as tile
from concourse import bacc
from concourse.bass_utils import run_bass_kernel_spmd
from concourse.masks import make_identity

F32 = mybir.dt.float32
F32R = mybir.dt.float32r
BF16 = mybir.dt.bfloat16
AF = mybir.ActivationFunctionType

EXPERT_DT = "bf16"  # "bf16" or "f32r" for expert matmuls (router is exact f32)

T, H, FF, E, TOPK = 2048, 1024, 512, 16, 4
NCORES, EPC = 8, 2  # cores, experts per core
TC = 512  # token chunk
KT = H // 128  # contraction tiles over H
FT = FF // 128  # contraction tiles over FF
HT = H // 128  # output tiles over H
NEG = -1.0e30


def moe_program(tc_ctx, io, n_tok, expert_dt=None):
    nc = tc_ctx.nc
    tc = tc_ctx
    nch = n_tok // TC
    nsub = TC // 128
    ed = BF16 if (expert_dt or EXPERT_DT) == "bf16" else F32R
    is_bf = ed == BF16

    def wcast(ap):
        return ap if is_bf else ap.bitcast(F32R)

    with (
        tc.tile_pool(name="const", bufs=1) as constp,
        tc.tile_pool(name="wpool", bufs=1) as wp_pool,
        tc.tile_pool(name="xtp", bufs=2 if (expert_dt or EXPERT_DT) == "bf16" else 1) as xtp,
        tc.tile_pool(name="xtrp", bufs=1) as xtrp,
        tc.tile_pool(name="rpool", bufs=2) as rpool,
        tc.tile_pool(name="cwp", bufs=2) as cwp,
        tc.tile_pool(name="silup", bufs=2) as silup,
        tc.tile_pool(name="actp", bufs=1) as actp,
        tc.tile_pool(name="actshp", bufs=2) as actshp,
        tc.tile_pool(name="stagep", bufs=2) as stagep,
        tc.tile_pool(name="pgu", bufs=2, space="PSUM") as pgu,
        tc.tile_pool(name="psgu", bufs=2, space="PSUM") as psgu,
        tc.tile_pool(name="pflex", bufs=2, space="PSUM") as pflex,
        tc.tile_pool(name="dramp", bufs=2, space="DRAM") as dramp,
    ):
        # ---- constants / weights (resident) ----
        ident = constp.tile([128, 128], F32)
        make_identity(nc, ident[:])
        onesb = constp.tile([128, 128], F32)
        nc.vector.memset(onesb[:], 1.0)
        gate_sb = constp.tile([128, KT, E], F32)
        nc.sync.dma_start(gate_sb[:], io["gate"])
        bias_sb = constp.tile([128, E], F32)
        nc.sync.dma_start(bias_sb[0:1, :], io["biasp"])

        # chunk-0 x load first so the router can start immediately
        xt_tiles = {}
        xt_tiles[0] = xtp.tile([128, KT, TC], F32, name="xt32", tag="xt32")
        nc.sync.dma_start(xt_tiles[0][:], io["xt"][:, :, 0:TC])

        # weight loads split so chunk-0 compute overlaps the rest
        w1sb = [
            wp_pool.tile([128, KT, 8, 128], ed, name=f"w1sb{e}") for e in range(EPC)
        ]
        w2sb = [
            wp_pool.tile([128, FT, HT, 128], ed, name=f"w2sb{e}") for e in range(EPC)
        ]
        for e in range(EPC):
            for j in range(FT):
                nc.sync.dma_start(
                    w1sb[e][:, :, 2 * j : 2 * j + 2, :],
                    wcast(io["w1t"][e][:, :, 2 * j : 2 * j + 2, :]),
                )
        sw1sb = wp_pool.tile([128, KT, 2, 64], ed)
        nc.sync.dma_start(sw1sb[:], wcast(io["sw1t"]))
        for e in range(EPC):
            nc.sync.dma_start(w2sb[e][:], wcast(io["w2t"][e]))
        sw2sb = wp_pool.tile([128, HT, 128], ed)
        nc.sync.dma_start(sw2sb[0:64, :, :], wcast(io["sw2t"]))

        # bias broadcast [1,E] -> [128,E]
        bps = pflex.tile([128, TC], F32, name="bps", tag="flex")
        nc.tensor.matmul(
            bps[:, 0:E], onesb[0:1, :], bias_sb[0:1, :], start=True, stop=True
        )
        biasbc = constp.tile([128, E], F32)
        nc.scalar.copy(biasbc[:], bps[:, 0:E])

        for c in range(nch):
            xt32 = xt_tiles.pop(c)
            if c + 1 < nch:  # prefetch next chunk's x
                xt_tiles[c + 1] = xtp.tile([128, KT, TC], F32, name="xt32", tag="xt32")
                nc.sync.dma_start(
                    xt_tiles[c + 1][:], io["xt"][:, :, (c + 1) * TC : (c + 2) * TC]
                )
            xtr = xtrp.tile([128, KT, TC], ed, name="xtr", tag="xtr")
            nc.scalar.copy(xtr[:], xt32[:])

            # ---- router (exact fp32) ----
            rps_full = pflex.tile([128, TC], F32, name="rps", tag="flex")
            rps = rps_full[0:16, :]
            for kt in range(KT):
                nc.tensor.matmul(
                    rps,
                    gate_sb[:, kt, :],
                    xt32[:, kt, :],
                    start=(kt == 0),
                    stop=(kt == KT - 1),
                )
            scoresTb = rpool.tile([128, TC], F32, name="scoresT", tag="scoresT")
            scoresT = scoresTb[0:16, :]
            nc.scalar.activation(scoresT, rps, AF.Sigmoid)

            cwTbs = [
                cwp.tile([128, TC], F32, name=f"cwT{e}", tag=f"cwT{e}")
                for e in range(EPC)
            ]
            for s in range(nsub):
                tps_full = pflex.tile([128, TC], F32, name="tps", tag="flex")
                tps = tps_full[:, 0:16]
                nc.tensor.transpose(
                    tps, scoresT[:, s * 128 : (s + 1) * 128], ident[0:16, 0:16]
                )
                scores = rpool.tile([128, E], F32, name="scores", tag="scores")
                nc.scalar.copy(scores[:], tps)
                biased = rpool.tile([128, E], F32, name="biased", tag="biased")
                nc.vector.tensor_add(biased[:], scores[:], biasbc[:])
                t8 = rpool.tile([128, 8], F32, name="t8", tag="t8")
                nc.vector.max(out=t8[:], in_=biased[:])
                nc.vector.memset(t8[:, TOPK:], NEG)
                zap = rpool.tile([128, E], F32, name="zap", tag="zap")
                nc.vector.match_replace(
                    out=zap[:], in_to_replace=t8[:], in_values=biased[:], imm_value=NEG
                )
                sel = rpool.tile([128, E], F32, name="sel", tag="sel")
                nc.vector.tensor_tensor(
                    sel[:], biased[:], zap[:], mybir.AluOpType.not_equal
                )
                cwu = rpool.tile([128, E], F32, name="cwu", tag="cwu")
                nc.vector.tensor_mul(cwu[:], scores[:], sel[:])
                den = rpool.tile([128, 1], F32, name="den", tag="den")
                nc.vector.reduce_sum(den[:], cwu[:], axis=mybir.AxisListType.X)
                rden = rpool.tile([128, 1], F32, name="rden", tag="rden")
                nc.vector.reciprocal(rden[:], den[:])
                cw = rpool.tile([128, EPC], F32, name="cw", tag="cw")
                nc.vector.tensor_scalar_mul(cw[:], cwu[:, 0:EPC], rden[:])
                for e in range(EPC):
                    ctp_full = pflex.tile([128, TC], F32, name="ctp", tag="flex")
                    ctp = ctp_full[0:1, 0:128]
                    nc.tensor.transpose(ctp, cw[:, e : e + 1], ident[:])
                    nc.scalar.copy(cwTbs[e][0:1, s * 128 : (s + 1) * 128], ctp)

            # cw broadcast across partitions via k=1 matmul
            cwbc = cwp.tile([128, EPC, TC], F32, name="cwbc", tag="cwbc")
            for e in range(EPC):
                bc = pflex.tile([128, TC], F32, name="bc", tag="flex")
                nc.tensor.matmul(
                    bc[:], onesb[0:1, :], cwTbs[e][0:1, :], start=True, stop=True
                )
                nc.scalar.copy(cwbc[:, e, :], bc[:])

            # ---- experts: w1 + swiglu (+ cw scale) ----
            actbs = []
            for e in range(EPC):
                actb = actp.tile([128, FT, TC], ed, name=f"actb{e}", tag=f"actb{e}")
                for j in range(FT):
                    gup = pgu.tile([128, 2, TC], F32, name="gup", tag="gu")
                    for half in range(2):
                        for kt in range(KT):
                            nc.tensor.matmul(
                                gup[:, half, :],
                                w1sb[e][:, kt, 2 * j + half, :],
                                xtr[:, kt, :],
                                start=(kt == 0),
                                stop=(kt == KT - 1),
                            )
                    sg = silup.tile([128, TC], F32, name="sg", tag="sg")
                    nc.scalar.activation(sg[:], gup[:, 0, :], AF.Sigmoid)
                    nc.vector.tensor_mul(sg[:], sg[:], gup[:, 0, :])
                    usc = silup.tile([128, TC], F32, name="usc", tag="usc")
                    nc.vector.tensor_mul(usc[:], gup[:, 1, :], cwbc[:, e, :])
                    nc.vector.tensor_mul(actb[:, j, :], sg[:], usc[:])
                actbs.append(actb)

            # ---- shared expert (TP slice: 64 f-rows) ----
            sgug_f = psgu.tile([128, TC], F32, name="sgug", tag="sgu")
            sguu_f = psgu.tile([128, TC], F32, name="sguu", tag="sgu")
            sgug, sguu = sgug_f[0:64, :], sguu_f[0:64, :]
            for kt in range(KT):
                nc.tensor.matmul(
                    sgug,
                    sw1sb[:, kt, 0, :],
                    xtr[:, kt, :],
                    start=(kt == 0),
                    stop=(kt == KT - 1),
                )
            for kt in range(KT):
                nc.tensor.matmul(
                    sguu,
                    sw1sb[:, kt, 1, :],
                    xtr[:, kt, :],
                    start=(kt == 0),
                    stop=(kt == KT - 1),
                )
            sgs_f = silup.tile([128, TC], F32, name="sgs", tag="sgs")
            sgs = sgs_f[0:64, :]
            nc.scalar.activation(sgs, sgug, AF.Sigmoid)
            nc.vector.tensor_mul(sgs, sgs, sgug)
            actsh_f = actshp.tile([128, TC], ed, name="actsh", tag="actsh")
            actsh = actsh_f[0:64, :]
            nc.vector.tensor_mul(actsh, sgs, sguu)

            # ---- w2: accumulate both experts + shared into partial [H, TC] ----
            partial = dramp.tile([H, TC], F32, name="partial", tag="partial")
            for ht in range(HT):
                wps = pflex.tile([128, TC], F32, name="wps", tag="flex")
                nk = 0
                for e in range(EPC):
                    for ft in range(FT):
                        nc.tensor.matmul(
                            wps[:],
                            w2sb[e][:, ft, ht, :],
                            actbs[e][:, ft, :],
                            start=(nk == 0),
                            stop=False,
                        )
                        nk += 1
                nc.tensor.matmul(
                    wps[:], sw2sb[0:64, ht, :], actsh, start=False, stop=True
                )
                stg = stagep.tile([128, TC], F32, name="stg", tag="stg")
                nc.scalar.copy(stg[:], wps[:])
                nc.sync.dma_start(partial[ht * 128 : (ht + 1) * 128, :], stg[:])

            # ---- combine across cores ----
            rso = dramp.tile([128, TC], F32, name="rso", tag="rso")
            nc.gpsimd.collective_compute(
                "ReduceScatter",
                mybir.AluOpType.add,
                replica_groups=[list(range(NCORES))],
                ins=[partial.opt()],
                outs=[rso.opt()],
            )
            nc.sync.dma_start(io["out"][:, c * TC : (c + 1) * TC], rso[:])


def build_nc(n_tok=T, expert_dt=None):
    ed_name = expert_dt or EXPERT_DT
    wdt = BF16 if ed_name == "bf16" else F32
    nc = bacc.Bacc("TRN2", target_bir_lowering=False, debug=False, num_devices=NCORES)
    io = {
        "xt": nc.dram_tensor("xt", [128, KT, n_tok], F32, kind="ExternalInput").ap(),
        "gate": nc.dram_tensor("gate", [128, KT, E], F32, kind="ExternalInput").ap(),
        "biasp": nc.dram_tensor("biasp", [1, E], F32, kind="ExternalInput").ap(),
        "w1t": [
            nc.dram_tensor(f"w1t{e}", [128, KT, 8, 128], wdt, kind="ExternalInput").ap()
            for e in range(EPC)
        ],
        "w2t": [
            nc.dram_tensor(f"w2t{e}", [128, FT, HT, 128], wdt, kind="ExternalInput").ap()
            for e in range(EPC)
        ],
        "sw1t": nc.dram_tensor("sw1t", [128, KT, 2, 64], wdt, kind="ExternalInput").ap(),
        "sw2t": nc.dram_tensor("sw2t", [64, HT, 128], wdt, kind="ExternalInput").ap(),
        "out": nc.dram_tensor("out", [128, n_tok], F32, kind="ExternalOutput").ap(),
    }
    with tile.TileContext(nc) as tcx:
        moe_program(tcx, io, n_tok, expert_dt=ed_name)
    nc.compile()
    return nc


def _sb_layout_kxm(a):
    """[Ktot, M] -> sbuf image [128, KT', M] with h-tiles on axis 1."""
    ktot, m = a.shape
    return np.ascontiguousarray(
        a.reshape(ktot // 128, 128, m).transpose(1, 0, 2)
    )


def prep_inputs(inputs, n_tok=T, expert_dt=None):
    """Host-side sharding/layout: returns per-core in_maps."""
    import ml_dtypes

    wnp = (
        np.dtype(ml_dtypes.bfloat16)
        if (expert_dt or EXPERT_DT) == "bf16"
        else np.float32
    )
    x = np.asarray(inputs["hidden_states"], dtype=np.float32)[:n_tok]
    gate_w = np.asarray(inputs["gate_w"], dtype=np.float32)
    ebias = np.asarray(inputs["expert_bias"], dtype=np.float32)
    w1 = np.asarray(inputs["w1"], dtype=np.float32)
    w2 = np.asarray(inputs["w2"], dtype=np.float32)
    sw1 = np.asarray(inputs["shared_w1"], dtype=np.float32)
    sw2 = np.asarray(inputs["shared_w2"], dtype=np.float32)

    xt = _sb_layout_kxm(np.ascontiguousarray(x.T))  # [128, KT, n_tok]

    in_maps = []
    for r in range(NCORES):
        perm = [2 * r, 2 * r + 1] + [e for e in range(E) if e not in (2 * r, 2 * r + 1)]
        gperm = gate_w[perm].T  # [H, E]
        m = {
            "xt": xt,
            "gate": _sb_layout_kxm(gperm).copy(),
            "biasp": ebias[perm][None, :].copy(),
            "sw1t": _sb_layout_kxm(
                np.concatenate(
                    [sw1.T[:, 64 * r : 64 * r + 64], sw1.T[:, FF + 64 * r : FF + 64 * r + 64]],
                    axis=1,
                )
            ).reshape(128, KT, 2, 64).astype(wnp),
            "sw2t": np.ascontiguousarray(sw2.T[64 * r : 64 * r + 64]).reshape(
                64, HT, 128
            ).astype(wnp),
        }
        for i, e in enumerate((2 * r, 2 * r + 1)):
            w1te = w1[e].T  # [H, 2FF] cols: g 0..FF-1, u FF..2FF-1
            inter = np.empty_like(w1te)
            for j in range(FT):
                inter[:, 256 * j : 256 * j + 128] = w1te[:, 128 * j : 128 * j + 128]
                inter[:, 256 * j + 128 : 256 * j + 256] = w1te[
                    :, FF + 128 * j : FF + 128 * j + 128
                ]
            m[f"w1t{i}"] = _sb_layout_kxm(inter).reshape(128, KT, 8, 128).astype(wnp)
            m[f"w2t{i}"] = _sb_layout_kxm(np.ascontiguousarray(w2[e].T)).reshape(
                128, FT, HT, 128
            ).astype(wnp)
        in_maps.append(m)
    return in_maps


def assemble_output(results, n_tok=T):
    shards = [results[r]["out"] for r in range(NCORES)]
    full = np.concatenate(shards, axis=0)  # [H, n_tok]
    return np.ascontiguousarray(full.T).astype(np.float32)


_NC_CACHE = {}


def kernel(**inputs):
    n_tok = int(np.asarray(inputs["hidden_states"]).shape[0])
    if n_tok not in _NC_CACHE:
        _NC_CACHE[n_tok] = build_nc(n_tok)
    nc = _NC_CACHE[n_tok]
    in_maps = prep_inputs(inputs, n_tok)
    res = run_bass_kernel_spmd(nc, in_maps, core_ids=list(range(NCORES)))
    return assemble_output(res.results, n_tok)


if __name__ == "__main__":
    import reference  # only when run manually alongside reference.py

    inp = reference.setup_inputs()
    out = kernel(**{k: np.asarray(v) for k, v in inp.items()})
    print("out", out.shape, out.dtype)
